# revision 1
# baseline (speedup 1.0000x reference)
"""Trainium2 Bass kernel for CusMultiHeadAttention.

Shapes (hardcoded): x (4,1024,1024) f32, bias (4,16,1024,1024) f32,
attention_mask (4,1024) i32, Wq/Wk/Wv (1024,1024), Wo (1024,1024), bo (1024,).

Sharding: 8 cores = 4 batches x 2 head-groups (8 heads each).
Wq/Wk/Wv column-parallel, Wo row-parallel (host sums the pair partials + bo).

Per-core pipeline (all "transposed" orientation, no on-device transposes):
  xT = x[b].T (host)                     -> SBUF (c_in on partitions)
  qT = (Wq'/8)^T @ xT, kT = Wk'^T @ xT   (feature on partitions, seq free)
  v  = x[b] @ Wv'                        (seq on partitions, feature free)
  v_aug[h] = [v[h] * mask | mask]        (mask folded into V + ones-column)
  sT[h,kt] = kT[h,kt].T @ qT[h] + biasT  (k on partitions, q free; biasT from host)
  pT = exp(sT)                           (no max subtraction; scores are O(5))
  o_aug[h] = sum_kt v_aug[h,kt].T @ pT[h,kt]   (rows 0..63 = o.T, row 64 = denom)
  oT[h] = o_aug[0:64] * bcast(1/denom)   (denom recip at p64 hops to p0 via a
                                          tiny SBUF DMA, then gpsimd broadcast)
  outp = sum_h oT[h].T @ Wo'[h]          (q on partitions) -> DRAM partial
"""

import sys

if "/opt/trn_rl_repo" not in sys.path:
    sys.path.insert(0, "/opt/trn_rl_repo")

import math
from contextlib import ExitStack

import numpy as np

import concourse.mybir as mybir
import concourse.tile as tile
from concourse import bacc
from concourse.alu_op_type import AluOpType
from concourse.bass_utils import run_bass_kernel_spmd

B, S, C_IN = 4, 1024, 1024
N_HEAD, C = 16, 64
N_CORES = 8
HG = 8  # heads per core
F = HG * C  # 512 local features
P = 128
KT = C_IN // P  # 8 contraction tiles for projections
ST = S // P  # 8 seq tiles
VW = C + 1  # 65: v columns + ones-column

f32 = mybir.dt.float32
bf16 = mybir.dt.bfloat16


def build_program(taps=False):
    nc = bacc.Bacc("TRN2", target_bir_lowering=False, debug=False,
                   num_devices=N_CORES)

    xT = nc.dram_tensor("xT", (C_IN, S), bf16, kind="ExternalInput").ap()
    wq = nc.dram_tensor("wq", (C_IN, F), bf16, kind="ExternalInput").ap()
    wk = nc.dram_tensor("wk", (C_IN, F), bf16, kind="ExternalInput").ap()
    wv = nc.dram_tensor("wv", (C_IN, F), bf16, kind="ExternalInput").ap()
    wo = nc.dram_tensor("wo", (F, C_IN), bf16, kind="ExternalInput").ap()
    biasT = nc.dram_tensor("biasT", (HG, S, S), bf16, kind="ExternalInput").ap()
    maskf = nc.dram_tensor("maskf", (S,), f32, kind="ExternalInput").ap()
    outp = nc.dram_tensor("outp", (S, C_IN), f32, kind="ExternalOutput").ap()
    if taps:
        dbg_qT = nc.dram_tensor("dbg_qT", (P, F // P, S), bf16,
                                kind="ExternalOutput").ap()
        dbg_kT = nc.dram_tensor("dbg_kT", (P, F // P, S), bf16,
                                kind="ExternalOutput").ap()
        dbg_v = nc.dram_tensor("dbg_v", (P, ST, HG * VW), bf16,
                               kind="ExternalOutput").ap()
        dbg_oT = nc.dram_tensor("dbg_oT", (C, HG, S), bf16,
                                kind="ExternalOutput").ap()
        dbg_pt = nc.dram_tensor("dbg_pt", (P, S), bf16,
                                kind="ExternalOutput").ap()
        dbg_rc = nc.dram_tensor("dbg_rc", (1, S), f32,
                                kind="ExternalOutput").ap()
        dbg_rcb = nc.dram_tensor("dbg_rcb", (C, S), f32,
                                 kind="ExternalOutput").ap()

    with tile.TileContext(nc) as tc:
        with ExitStack() as ctx:
            persist = ctx.enter_context(tc.tile_pool(name="persist", bufs=1))
            mask_sb = persist.tile([P, ST], f32)
            nc.sync.dma_start(mask_sb[:], maskf.rearrange("(t p) -> p t", p=P))
            ones_sb = persist.tile([P, HG, 1], f32)
            nc.vector.memset(ones_sb[:], 1.0)
            v_sb = persist.tile([P, ST, HG * VW], bf16)
            qT_sb = persist.tile([P, F // P, S], bf16)
            kT_sb = persist.tile([P, F // P, S], bf16)
            # per-head rows at partitions 0..63 (matmul needs equal base
            # partition for lhsT and rhs; oT lives at partitions 0..63)
            wo_sb = persist.tile([C, HG, C_IN], bf16)
            nc.sync.dma_start(
                wo_sb[:], wo.rearrange("(h j) n -> j h n", j=C))

            # ---- phase A: projections (xT/wq/wk/wv live only here) ----
            with tc.tile_pool(name="phaseA", bufs=1) as pa, \
                 tc.tile_pool(name="psProj", bufs=2, space="PSUM") as psProj, \
                 tc.tile_pool(name="psV", bufs=2, space="PSUM") as psV:
                xT_sb = pa.tile([P, KT, S], bf16)
                wq_sb = pa.tile([P, KT, F], bf16)
                wk_sb = pa.tile([P, KT, F], bf16)
                wv_sb = pa.tile([P, KT, F], bf16)
                for kt in range(KT):
                    nc.sync.dma_start(
                        xT_sb[:, kt, :],
                        xT[kt * P:(kt + 1) * P, :])
                    nc.sync.dma_start(
                        wq_sb[:, kt, :],
                        wq[kt * P:(kt + 1) * P, :])
                    nc.sync.dma_start(
                        wk_sb[:, kt, :],
                        wk[kt * P:(kt + 1) * P, :])
                    nc.sync.dma_start(
                        wv_sb[:, kt, :],
                        wv[kt * P:(kt + 1) * P, :])

                # qT, kT: (feature on partitions, seq free)
                for mt in range(F // P):
                    for w_sb, dst in ((wq_sb, qT_sb), (wk_sb, kT_sb)):
                        ps = psProj.tile([P, S], f32, name="ps_proj")
                        for nh in range(2):
                            for kt in range(KT):
                                nc.tensor.matmul(
                                    ps[:, nh * 512:(nh + 1) * 512],
                                    w_sb[:, kt, mt * P:(mt + 1) * P],
                                    xT_sb[:, kt, nh * 512:(nh + 1) * 512],
                                    start=(kt == 0), stop=(kt == KT - 1))
                        nc.scalar.copy(dst[:, mt, :], ps[:])

                # v natural (seq on partitions), mask+ones folded
                for mt in range(ST):
                    psv = psV.tile([P, F], f32, name="psv")
                    for kt in range(KT):
                        nc.tensor.matmul(
                            psv[:],
                            xT_sb[:, kt, mt * P:(mt + 1) * P],
                            wv_sb[:, kt, :],
                            start=(kt == 0), stop=(kt == KT - 1))
                    m_col = mask_sb[:, mt:mt + 1]
                    v_view = v_sb[:, mt, :].rearrange("p (h c) -> p h c", c=VW)
                    nc.vector.tensor_scalar_mul(
                        v_view[:, :, 0:C],
                        psv.rearrange("p (h c) -> p h c", c=C), m_col)
                    nc.vector.tensor_scalar_mul(
                        v_view[:, :, C:C + 1], ones_sb[:], m_col)

            # ---- phase B: attention ----
            oT_pool = ctx.enter_context(tc.tile_pool(name="oTp", bufs=1))
            oT_sb = oT_pool.tile([C, HG, S], bf16)
            with tc.tile_pool(name="bias", bufs=8) as bias_pool, \
                 tc.tile_pool(name="pT", bufs=4) as pT_pool, \
                 tc.tile_pool(name="rc", bufs=2) as rc_pool, \
                 tc.tile_pool(name="rc0", bufs=2) as rc0_pool, \
                 tc.tile_pool(name="rcb", bufs=2) as rcb_pool, \
                 tc.tile_pool(name="psS", bufs=2, space="PSUM") as psS, \
                 tc.tile_pool(name="psO", bufs=2, space="PSUM") as psO:

                for h in range(HG):
                    po = (h % 2) * C  # partition offset of head in qT/kT
                    mt_h = h // 2
                    kT_h = kT_sb[po:po + C, mt_h, :]
                    qT_h = qT_sb[po:po + C, mt_h, :]
                    oaps = psO.tile([VW, S], f32, name="oaug")
                    for kt in range(ST):
                        ps_s = psS.tile([P, S], f32, name="ps_s")
                        for nh in range(2):
                            nc.tensor.matmul(
                                ps_s[:, nh * 512:(nh + 1) * 512],
                                kT_h[:, kt * P:(kt + 1) * P],
                                qT_h[:, nh * 512:(nh + 1) * 512],
                                start=True, stop=True)
                        bt = bias_pool.tile([P, S], bf16, name="bt")
                        nc.sync.dma_start(bt[:],
                                          biasT[h, kt * P:(kt + 1) * P, :])
                        nc.vector.tensor_tensor(ps_s[:], ps_s[:], bt[:],
                                                AluOpType.add)
                        pt = pT_pool.tile([P, S], bf16, name="pt")
                        nc.scalar.activation(pt[:], ps_s[:],
                                             mybir.ActivationFunctionType.Exp)
                        if taps and h == 0 and kt == 0:
                            nc.sync.dma_start(dbg_pt, pt[:])
                        for nh in range(2):
                            nc.tensor.matmul(
                                oaps[:, nh * 512:(nh + 1) * 512],
                                v_sb[:, kt, h * VW:(h + 1) * VW],
                                pt[:, nh * 512:(nh + 1) * 512],
                                start=(kt == 0), stop=(kt == ST - 1))
                    # denom row sits at psum partition 64: copy to SBUF,
                    # hop to p0 via tiny SBUF DMA, recip, broadcast.
                    rc = rc_pool.tile([P, S], f32, name="rc")
                    nc.scalar.copy(rc[C:C + 1, :], oaps[C:C + 1, :])
                    rc0 = rc0_pool.tile([1, S], f32, name="rc0")
                    nc.sync.dma_start(rc0[:], rc[C:C + 1, :])
                    rcv = rc0_pool.tile([1, S], f32, name="rcv", tag="rcv")
                    nc.vector.reciprocal_approx_fast(rcv[:], rc0[:])
                    rcb = rcb_pool.tile([C, S], f32, name="rcb")
                    nc.gpsimd.partition_broadcast(rcb[:], rcv[:])
                    if taps and h == 0:
                        nc.sync.dma_start(dbg_rc[:], rcv[:])
                        nc.sync.dma_start(dbg_rcb[:], rcb[:])
                    nc.vector.tensor_mul(oT_sb[:, h, :], oaps[0:C, :], rcb[:])

                if taps:
                    nc.sync.dma_start(dbg_qT, qT_sb[:])
                    nc.sync.dma_start(dbg_kT, kT_sb[:])
                    nc.sync.dma_start(dbg_v, v_sb[:])
                    nc.sync.dma_start(dbg_oT, oT_sb[:])

            # ---- output projection (row-parallel partial) ----
            with tc.tile_pool(name="outsb", bufs=3) as out_pool, \
                 tc.tile_pool(name="psOut", bufs=2, space="PSUM") as psOut:
                for qt in range(ST):
                    for nh in range(2):
                        pso = psOut.tile([P, 512], f32, name="pso")
                        for h in range(HG):
                            nc.tensor.matmul(
                                pso[:],
                                oT_sb[:, h, qt * P:(qt + 1) * P],
                                wo_sb[:, h, nh * 512:(nh + 1) * 512],
                                start=(h == 0), stop=(h == HG - 1))
                        osb = out_pool.tile([P, 512], f32, name="osb")
                        nc.scalar.copy(osb[:], pso[:])
                        nc.sync.dma_start(
                            outp[qt * P:(qt + 1) * P,
                                 nh * 512:(nh + 1) * 512],
                            osb[:])

    nc.compile()
    return nc


def make_in_maps(x, bias, attention_mask, Wq, Wk, Wv, Wo):
    import ml_dtypes
    bf = ml_dtypes.bfloat16
    scale = 1.0 / math.sqrt(C)
    wq_scaled = (np.asarray(Wq) * scale).astype(bf)
    x = np.asarray(x)
    bias = np.asarray(bias)
    wk16 = np.asarray(Wk).astype(bf)
    wv16 = np.asarray(Wv).astype(bf)
    wo16 = np.asarray(Wo).astype(bf)
    in_maps = []
    for c in range(N_CORES):
        b, hg = c // 2, c % 2
        fs = slice(hg * F, (hg + 1) * F)
        in_maps.append({
            "xT": np.ascontiguousarray(x[b].T.astype(bf)),
            "wq": np.ascontiguousarray(wq_scaled[:, fs]),
            "wk": np.ascontiguousarray(wk16[:, fs]),
            "wv": np.ascontiguousarray(wv16[:, fs]),
            "wo": np.ascontiguousarray(wo16[fs, :]),
            "biasT": np.ascontiguousarray(
                bias[b, hg * HG:(hg + 1) * HG].transpose(0, 2, 1).astype(bf)),
            "maskf": np.asarray(attention_mask)[b].astype(np.float32),
        })
    return in_maps


_NC_CACHE = []


def get_program():
    if not _NC_CACHE:
        _NC_CACHE.append(build_program())
    return _NC_CACHE[0]


def run(in_maps, trace=False, **kw):
    nc = get_program()
    return run_bass_kernel_spmd(nc, in_maps, core_ids=list(range(N_CORES)),
                                trace=trace, **kw)


def kernel(x, bias, attention_mask, Wq, Wk, Wv, Wo, bo):
    in_maps = make_in_maps(x, bias, attention_mask, Wq, Wk, Wv, Wo)
    res = run(in_maps)
    out = np.empty((B, S, C_IN), dtype=np.float32)
    for b in range(B):
        out[b] = (res.results[2 * b]["outp"] + res.results[2 * b + 1]["outp"]
                  + np.asarray(bo).astype(np.float32))
    return out



# revision 3
# speedup vs baseline: 1.0459x; 1.0459x over previous
"""Trainium2 Bass kernel for CusMultiHeadAttention.

Shapes (hardcoded): x (4,1024,1024) f32, bias (4,16,1024,1024) f32,
attention_mask (4,1024) i32, Wq/Wk/Wv (1024,1024), Wo (1024,1024), bo (1024,).

Sharding: 8 cores = 4 batches x 2 head-groups (8 heads each).
Wq/Wk/Wv column-parallel, Wo row-parallel (host sums the pair partials + bo).

Differences from the first working version (292us), all aimed at keeping the
PE tensor engine dense so the HAM clock gate stays at 2.4GHz:
  - bias is sent as exp(bias) (bf16); the f32 psum bias-add (DVE 1x mode)
    becomes a bf16 x bf16 multiply after exp (DVE 2x_1P mode).
  - phase B is software-pipelined: scores for chain i issue ahead of the
    AV matmuls for chain i-2, so the PE queue never head-of-line blocks
    on the exp/mul chain.
  - V projection runs first (kt-outer, 4 psum groups x 2 passes) so the PE
    starts as soon as the first xT/wv chunks land; QK projections follow and
    phase B scores chain directly behind them.
  - output projection packs head pairs on 128 partitions (K=128 matmuls);
    odd heads' oT is partition-shifted 0:64 -> 64:128 via SBUF-SBUF DMA.
  - outp partial is bf16 (host accumulates in f32).
"""

import sys

if "/opt/trn_rl_repo" not in sys.path:
    sys.path.insert(0, "/opt/trn_rl_repo")

import math
from contextlib import ExitStack

import numpy as np

import concourse.mybir as mybir
import concourse.tile as tile
from concourse import bacc
from concourse.alu_op_type import AluOpType
from concourse.bass_utils import run_bass_kernel_spmd

B, S, C_IN = 4, 1024, 1024
N_HEAD, C = 16, 64
N_CORES = 8
HG = 8  # heads per core
F = HG * C  # 512 local features
P = 128
KT = C_IN // P  # 8 contraction tiles for projections
ST = S // P  # 8 seq tiles
VW = C + 1  # 65: v columns + ones-column
NHW = 512  # psum bank width in f32

f32 = mybir.dt.float32
bf16 = mybir.dt.bfloat16

POOL_MUL_EVERY = 6  # every Nth expb-mul goes to gpsimd instead of DVE


def build_program():
    nc = bacc.Bacc("TRN2", target_bir_lowering=False, debug=False,
                   num_devices=N_CORES)

    xT = nc.dram_tensor("xT", (C_IN, S), bf16, kind="ExternalInput").ap()
    wq = nc.dram_tensor("wq", (C_IN, F), bf16, kind="ExternalInput").ap()
    wk = nc.dram_tensor("wk", (C_IN, F), bf16, kind="ExternalInput").ap()
    wv = nc.dram_tensor("wv", (C_IN, F), bf16, kind="ExternalInput").ap()
    wo = nc.dram_tensor("wo", (F, C_IN), bf16, kind="ExternalInput").ap()
    expb = nc.dram_tensor("expb", (HG, S, S), bf16, kind="ExternalInput").ap()
    maskf = nc.dram_tensor("maskf", (S,), f32, kind="ExternalInput").ap()
    outp = nc.dram_tensor("outp", (S, C_IN), bf16, kind="ExternalOutput").ap()

    with tile.TileContext(nc) as tc:
        with ExitStack() as ctx:
            persist = ctx.enter_context(tc.tile_pool(name="persist", bufs=1))
            mask_sb = persist.tile([P, ST], f32)
            nc.sync.dma_start(mask_sb[:], maskf.rearrange("(t p) -> p t", p=P))
            ones_sb = persist.tile([P, HG, 1], f32)
            nc.vector.memset(ones_sb[:], 1.0)
            v_sb = persist.tile([P, ST, HG * VW], bf16)
            qT_sb = persist.tile([P, F // P, S], bf16)
            kT_sb = persist.tile([P, F // P, S], bf16)
            # head-pair rows: partitions 0:64 = even head, 64:128 = odd head
            # of pair hp; matches wo rows hp*128:(hp+1)*128.
            wo2_sb = persist.tile([P, F // P, C_IN], bf16)
            nc.sync.dma_start(
                wo2_sb[:], wo.rearrange("(h j) n -> j h n", j=P))
            oT2_sb = persist.tile([P, F // P, S], bf16)

            # expb pool opened early so its DMAs prefetch during phase A
            expb_pool = ctx.enter_context(tc.tile_pool(name="expb", bufs=10))

            # ---- phase A: projections ----
            with tc.tile_pool(name="phaseA", bufs=1) as pa:
                xT_sb = pa.tile([P, KT, S], bf16)
                wv_sb = pa.tile([P, KT, F], bf16)
                wq_sb = pa.tile([P, KT, F], bf16)
                wk_sb = pa.tile([P, KT, F], bf16)
                for kt in range(KT):
                    nc.sync.dma_start(xT_sb[:, kt, :], xT[kt * P:(kt + 1) * P, :])
                    nc.sync.dma_start(wv_sb[:, kt, :], wv[kt * P:(kt + 1) * P, :])
                for kt in range(KT):
                    nc.sync.dma_start(wq_sb[:, kt, :], wq[kt * P:(kt + 1) * P, :])
                    nc.sync.dma_start(wk_sb[:, kt, :], wk[kt * P:(kt + 1) * P, :])

                # v natural (seq on partitions), kt-outer so the PE starts on
                # the first loaded chunk; 2 passes of 4 seq-tiles (4 psum banks)
                with tc.tile_pool(name="psV", bufs=1, space="PSUM") as psV:
                    for half in range(2):
                        mts = range(half * 4, half * 4 + 4)
                        psv_t = {mt: psV.tile([P, F], f32, name=f"psv{mt % 4}")
                                 for mt in mts}
                        for kt in range(KT):
                            for mt in mts:
                                nc.tensor.matmul(
                                    psv_t[mt][:],
                                    xT_sb[:, kt, mt * P:(mt + 1) * P],
                                    wv_sb[:, kt, :],
                                    start=(kt == 0), stop=(kt == KT - 1))
                        for mt in mts:
                            m_col = mask_sb[:, mt:mt + 1]
                            v_view = v_sb[:, mt, :].rearrange(
                                "p (h c) -> p h c", c=VW)
                            nc.vector.tensor_scalar_mul(
                                v_view[:, :, 0:C],
                                psv_t[mt].rearrange("p (h c) -> p h c", c=C),
                                m_col)
                            nc.vector.tensor_scalar_mul(
                                v_view[:, :, C:C + 1], ones_sb[:], m_col)

                # qT, kT (feature on partitions, seq free)
                with tc.tile_pool(name="psProj", bufs=2, space="PSUM") as psProj:
                    for mt in range(F // P):
                        for w_sb, dst in ((wq_sb, qT_sb), (wk_sb, kT_sb)):
                            ps = psProj.tile([P, S], f32, name="ps_proj")
                            for nh in range(2):
                                for kt in range(KT):
                                    nc.tensor.matmul(
                                        ps[:, nh * NHW:(nh + 1) * NHW],
                                        w_sb[:, kt, mt * P:(mt + 1) * P],
                                        xT_sb[:, kt, nh * NHW:(nh + 1) * NHW],
                                        start=(kt == 0), stop=(kt == KT - 1))
                            nc.scalar.copy(dst[:, mt, :], ps[:])

            # ---- phase B: attention, software-pipelined chains ----
            with tc.tile_pool(name="pS", bufs=3) as es_pool, \
                 tc.tile_pool(name="pT", bufs=6) as pt_pool, \
                 tc.tile_pool(name="rc", bufs=2) as rc_pool, \
                 tc.tile_pool(name="rc0", bufs=4) as rc0_pool, \
                 tc.tile_pool(name="rcb", bufs=2) as rcb_pool, \
                 tc.tile_pool(name="oshift", bufs=2) as oshift_pool, \
                 tc.tile_pool(name="psS", bufs=2, space="PSUM") as psS, \
                 tc.tile_pool(name="psO", bufs=2, space="PSUM") as psO:

                pts = {}
                oaps_t = {}

                def emit_scores(i, h, kt):
                    po = (h % 2) * C
                    mt_h = h // 2
                    eb = expb_pool.tile([P, S], bf16, name="eb")
                    nc.sync.dma_start(eb[:], expb[h, kt * P:(kt + 1) * P, :])
                    ps_s = psS.tile([P, S], f32, name="ps_s")
                    for nh in range(2):
                        nc.tensor.matmul(
                            ps_s[:, nh * NHW:(nh + 1) * NHW],
                            kT_sb[po:po + C, mt_h, kt * P:(kt + 1) * P],
                            qT_sb[po:po + C, mt_h, nh * NHW:(nh + 1) * NHW],
                            start=True, stop=True)
                    es = es_pool.tile([P, S], bf16, name="es")
                    nc.scalar.activation(es[:], ps_s[:],
                                         mybir.ActivationFunctionType.Exp)
                    pt = pt_pool.tile([P, S], bf16, name="pt")
                    eng = nc.gpsimd if (i % POOL_MUL_EVERY
                                        == POOL_MUL_EVERY - 1) else nc.vector
                    eng.tensor_mul(pt[:], es[:], eb[:])
                    pts[(h, kt)] = pt

                def emit_av(h, kt):
                    if kt == 0:
                        oaps_t[h] = psO.tile([VW, S], f32, name="oaug")
                    oaps = oaps_t[h]
                    pt = pts.pop((h, kt))
                    for nh in range(2):
                        nc.tensor.matmul(
                            oaps[:, nh * NHW:(nh + 1) * NHW],
                            v_sb[:, kt, h * VW:(h + 1) * VW],
                            pt[:, nh * NHW:(nh + 1) * NHW],
                            start=(kt == 0), stop=(kt == ST - 1))
                    if kt == ST - 1:
                        emit_norm(h)

                def emit_norm(h):
                    oaps = oaps_t.pop(h)
                    # denom row at psum partition 64: copy to SBUF, hop to p0
                    # via tiny SBUF DMA, recip, broadcast to 64 partitions.
                    rc = rc_pool.tile([VW, S], f32, name="rc")
                    nc.vector.tensor_copy(rc[C:C + 1, :], oaps[C:C + 1, :])
                    rc0 = rc0_pool.tile([1, S], f32, name="rc0")
                    nc.sync.dma_start(rc0[:], rc[C:C + 1, :])
                    rcv = rc0_pool.tile([1, S], f32, name="rcv", tag="rcv")
                    nc.vector.reciprocal_approx_fast(rcv[:], rc0[:])
                    rcb = rcb_pool.tile([C, S], f32, name="rcb")
                    nc.gpsimd.partition_broadcast(rcb[:], rcv[:])
                    hp = h // 2
                    if h % 2 == 0:
                        nc.vector.tensor_mul(oT2_sb[0:C, hp, :],
                                             oaps[0:C, :], rcb[:])
                    else:
                        oTs = oshift_pool.tile([C, S], bf16, name="oTs")
                        nc.vector.tensor_mul(oTs[:], oaps[0:C, :], rcb[:])
                        nc.sync.dma_start(oT2_sb[C:P, hp, :], oTs[:])

                chains = [(h, kt) for h in range(HG) for kt in range(ST)]
                LAG = 2
                for i, (h, kt) in enumerate(chains):
                    emit_scores(i, h, kt)
                    if i >= LAG:
                        emit_av(*chains[i - LAG])
                for j in range(len(chains) - LAG, len(chains)):
                    emit_av(*chains[j])

            # ---- phase C: output projection (row-parallel partial) ----
            with tc.tile_pool(name="outsb", bufs=3) as out_pool, \
                 tc.tile_pool(name="psOut", bufs=2, space="PSUM") as psOut:
                for qt in range(ST):
                    for nh in range(2):
                        pso = psOut.tile([P, NHW], f32, name="pso")
                        for hp in range(F // P):
                            nc.tensor.matmul(
                                pso[:],
                                oT2_sb[:, hp, qt * P:(qt + 1) * P],
                                wo2_sb[:, hp, nh * NHW:(nh + 1) * NHW],
                                start=(hp == 0), stop=(hp == F // P - 1))
                        osb = out_pool.tile([P, NHW], bf16, name="osb")
                        nc.scalar.copy(osb[:], pso[:])
                        nc.sync.dma_start(
                            outp[qt * P:(qt + 1) * P,
                                 nh * NHW:(nh + 1) * NHW],
                            osb[:])

    nc.compile()
    return nc


def make_in_maps(x, bias, attention_mask, Wq, Wk, Wv, Wo):
    import ml_dtypes
    bf = ml_dtypes.bfloat16
    scale = 1.0 / math.sqrt(C)
    wq_scaled = (np.asarray(Wq) * scale).astype(bf)
    x = np.asarray(x)
    bias = np.asarray(bias)
    wk16 = np.asarray(Wk).astype(bf)
    wv16 = np.asarray(Wv).astype(bf)
    wo16 = np.asarray(Wo).astype(bf)
    in_maps = []
    for c in range(N_CORES):
        b, hg = c // 2, c % 2
        fs = slice(hg * F, (hg + 1) * F)
        in_maps.append({
            "xT": np.ascontiguousarray(x[b].T.astype(bf)),
            "wq": np.ascontiguousarray(wq_scaled[:, fs]),
            "wk": np.ascontiguousarray(wk16[:, fs]),
            "wv": np.ascontiguousarray(wv16[:, fs]),
            "wo": np.ascontiguousarray(wo16[fs, :]),
            "expb": np.ascontiguousarray(
                np.exp(bias[b, hg * HG:(hg + 1) * HG].astype(np.float32))
                .transpose(0, 2, 1).astype(bf)),
            "maskf": np.asarray(attention_mask)[b].astype(np.float32),
        })
    return in_maps


_NC_CACHE = []


def get_program():
    if not _NC_CACHE:
        _NC_CACHE.append(build_program())
    return _NC_CACHE[0]


def run(in_maps, trace=False, **kw):
    nc = get_program()
    return run_bass_kernel_spmd(nc, in_maps, core_ids=list(range(N_CORES)),
                                trace=trace, **kw)


def kernel(x, bias, attention_mask, Wq, Wk, Wv, Wo, bo):
    in_maps = make_in_maps(x, bias, attention_mask, Wq, Wk, Wv, Wo)
    res = run(in_maps)
    out = np.empty((B, S, C_IN), dtype=np.float32)
    for b in range(B):
        out[b] = (res.results[2 * b]["outp"].astype(np.float32)
                  + res.results[2 * b + 1]["outp"].astype(np.float32)
                  + np.asarray(bo).astype(np.float32))
    return out


# revision 5
# speedup vs baseline: 1.0562x; 1.0098x over previous
"""Trainium2 Bass kernel for CusMultiHeadAttention.

Shapes (hardcoded): x (4,1024,1024) f32, bias (4,16,1024,1024) f32,
attention_mask (4,1024) i32, Wq/Wk/Wv (1024,1024), Wo (1024,1024), bo (1024,).

Sharding: 8 cores = 4 batches x 2 head-groups (8 heads each).
Wq/Wk/Wv column-parallel, Wo row-parallel (host sums the pair partials + bo).

Differences from the first working version (292us), all aimed at keeping the
PE tensor engine dense so the HAM clock gate stays at 2.4GHz:
  - bias is sent as exp(bias) (bf16); the f32 psum bias-add (DVE 1x mode)
    becomes a bf16 x bf16 multiply after exp (DVE 2x_1P mode).
  - phase B is software-pipelined: scores for chain i issue ahead of the
    AV matmuls for chain i-2, so the PE queue never head-of-line blocks
    on the exp/mul chain.
  - V projection runs first (kt-outer, 4 psum groups x 2 passes) so the PE
    starts as soon as the first xT/wv chunks land; QK projections follow and
    phase B scores chain directly behind them.
  - output projection packs head pairs on 128 partitions (K=128 matmuls);
    odd heads' oT is partition-shifted 0:64 -> 64:128 via SBUF-SBUF DMA.
  - outp partial is bf16 (host accumulates in f32).
"""

import sys

if "/opt/trn_rl_repo" not in sys.path:
    sys.path.insert(0, "/opt/trn_rl_repo")

import math
from contextlib import ExitStack

import numpy as np

import concourse.mybir as mybir
import concourse.tile as tile
from concourse import bacc
from concourse.alu_op_type import AluOpType
from concourse.bass_utils import run_bass_kernel_spmd

B, S, C_IN = 4, 1024, 1024
N_HEAD, C = 16, 64
N_CORES = 8
HG = 8  # heads per core
F = HG * C  # 512 local features
P = 128
KT = C_IN // P  # 8 contraction tiles for projections
ST = S // P  # 8 seq tiles
VW = C + 1  # 65: v columns + ones-column
NHW = 512  # psum bank width in f32

f32 = mybir.dt.float32
bf16 = mybir.dt.bfloat16

POOL_MUL_EVERY = 6  # every Nth expb-mul goes to gpsimd instead of DVE


def build_program():
    nc = bacc.Bacc("TRN2", target_bir_lowering=False, debug=False,
                   num_devices=N_CORES)

    xT = nc.dram_tensor("xT", (C_IN, S), bf16, kind="ExternalInput").ap()
    wq = nc.dram_tensor("wq", (C_IN, F), bf16, kind="ExternalInput").ap()
    wk = nc.dram_tensor("wk", (C_IN, F), bf16, kind="ExternalInput").ap()
    wv = nc.dram_tensor("wv", (C_IN, F), bf16, kind="ExternalInput").ap()
    wo = nc.dram_tensor("wo", (F, C_IN), bf16, kind="ExternalInput").ap()
    expb = nc.dram_tensor("expb", (HG, S, S), bf16, kind="ExternalInput").ap()
    maskf = nc.dram_tensor("maskf", (S,), f32, kind="ExternalInput").ap()
    outp = nc.dram_tensor("outp", (S, C_IN), bf16, kind="ExternalOutput").ap()

    with tile.TileContext(nc) as tc:
        with ExitStack() as ctx:
            persist = ctx.enter_context(tc.tile_pool(name="persist", bufs=1))
            mask_sb = persist.tile([P, ST], f32)
            nc.sync.dma_start(mask_sb[:], maskf.rearrange("(t p) -> p t", p=P))
            ones_sb = persist.tile([P, HG, 1], f32)
            nc.vector.memset(ones_sb[:], 1.0)
            v_sb = persist.tile([P, ST, HG * VW], bf16)
            qT_sb = persist.tile([P, F // P, S], bf16)
            kT_sb = persist.tile([P, F // P, S], bf16)
            # head-pair rows: partitions 0:64 = even head, 64:128 = odd head
            # of pair hp; matches wo rows hp*128:(hp+1)*128.
            wo2_sb = persist.tile([P, F // P, C_IN], bf16)
            nc.sync.dma_start(
                wo2_sb[:], wo.rearrange("(h j) n -> j h n", j=P))
            oT2_sb = persist.tile([P, F // P, S], bf16)

            # expb pool opened early so its DMAs prefetch during phase A
            expb_pool = ctx.enter_context(tc.tile_pool(name="expb", bufs=10))

            # ---- phase A: projections ----
            with tc.tile_pool(name="phaseA", bufs=1) as pa:
                xT_sb = pa.tile([P, KT, S], bf16)
                wv_sb = pa.tile([P, KT, F], bf16)
                wq_sb = pa.tile([P, KT, F], bf16)
                wk_sb = pa.tile([P, KT, F], bf16)
                for kt in range(KT):
                    nc.sync.dma_start(xT_sb[:, kt, :], xT[kt * P:(kt + 1) * P, :])
                    nc.sync.dma_start(wv_sb[:, kt, :], wv[kt * P:(kt + 1) * P, :])
                for kt in range(KT):
                    nc.sync.dma_start(wq_sb[:, kt, :], wq[kt * P:(kt + 1) * P, :])
                    nc.sync.dma_start(wk_sb[:, kt, :], wk[kt * P:(kt + 1) * P, :])

                # v natural (seq on partitions), kt-outer so the PE starts on
                # the first loaded chunk; 2 passes of 4 seq-tiles (4 psum banks)
                with tc.tile_pool(name="psV", bufs=1, space="PSUM") as psV:
                    for half in range(2):
                        mts = range(half * 4, half * 4 + 4)
                        psv_t = {mt: psV.tile([P, F], f32, name=f"psv{mt % 4}")
                                 for mt in mts}
                        for kt in range(KT):
                            for mt in mts:
                                nc.tensor.matmul(
                                    psv_t[mt][:],
                                    xT_sb[:, kt, mt * P:(mt + 1) * P],
                                    wv_sb[:, kt, :],
                                    start=(kt == 0), stop=(kt == KT - 1))
                        for mt in mts:
                            m_col = mask_sb[:, mt:mt + 1]
                            v_view = v_sb[:, mt, :].rearrange(
                                "p (h c) -> p h c", c=VW)
                            nc.vector.tensor_scalar_mul(
                                v_view[:, :, 0:C],
                                psv_t[mt].rearrange("p (h c) -> p h c", c=C),
                                m_col)
                            nc.vector.tensor_scalar_mul(
                                v_view[:, :, C:C + 1], ones_sb[:], m_col)

                # qT, kT (feature on partitions, seq free)
                with tc.tile_pool(name="psProj", bufs=2, space="PSUM") as psProj:
                    for mt in range(F // P):
                        for w_sb, dst in ((wq_sb, qT_sb), (wk_sb, kT_sb)):
                            ps = psProj.tile([P, S], f32, name="ps_proj")
                            for nh in range(2):
                                for kt in range(KT):
                                    nc.tensor.matmul(
                                        ps[:, nh * NHW:(nh + 1) * NHW],
                                        w_sb[:, kt, mt * P:(mt + 1) * P],
                                        xT_sb[:, kt, nh * NHW:(nh + 1) * NHW],
                                        start=(kt == 0), stop=(kt == KT - 1))
                            nc.scalar.copy(dst[:, mt, :], ps[:])

            # ---- phase B: attention, software-pipelined chains ----
            with tc.tile_pool(name="pS", bufs=3) as es_pool, \
                 tc.tile_pool(name="pT", bufs=6) as pt_pool, \
                 tc.tile_pool(name="rc", bufs=2) as rc_pool, \
                 tc.tile_pool(name="rc0", bufs=4) as rc0_pool, \
                 tc.tile_pool(name="rcb", bufs=2) as rcb_pool, \
                 tc.tile_pool(name="oshift", bufs=2) as oshift_pool, \
                 tc.tile_pool(name="psS", bufs=2, space="PSUM") as psS, \
                 tc.tile_pool(name="psO", bufs=2, space="PSUM") as psO:

                pts = {}
                oaps_t = {}
                events = {}  # chain index -> [callables] staggered norm stages

                def emit_scores(i, h, kt):
                    po = (h % 2) * C
                    mt_h = h // 2
                    eb = expb_pool.tile([P, S], bf16, name="eb")
                    nc.sync.dma_start(eb[:], expb[h, kt * P:(kt + 1) * P, :])
                    ps_s = psS.tile([P, S], f32, name="ps_s")
                    for nh in range(2):
                        nc.tensor.matmul(
                            ps_s[:, nh * NHW:(nh + 1) * NHW],
                            kT_sb[po:po + C, mt_h, kt * P:(kt + 1) * P],
                            qT_sb[po:po + C, mt_h, nh * NHW:(nh + 1) * NHW],
                            start=True, stop=True)
                    es = es_pool.tile([P, S], bf16, name="es")
                    nc.scalar.activation(es[:], ps_s[:],
                                         mybir.ActivationFunctionType.Exp)
                    pt = pt_pool.tile([P, S], bf16, name="pt")
                    eng = nc.gpsimd if (i % POOL_MUL_EVERY
                                        == POOL_MUL_EVERY - 1) else nc.vector
                    eng.tensor_mul(pt[:], es[:], eb[:])
                    pts[(h, kt)] = pt

                def emit_av(i, h, kt):
                    if kt == 0:
                        oaps_t[h] = psO.tile([VW, S], f32, name="oaug")
                    oaps = oaps_t[h]
                    pt = pts.pop((h, kt))
                    for nh in range(2):
                        nc.tensor.matmul(
                            oaps[:, nh * NHW:(nh + 1) * NHW],
                            v_sb[:, kt, h * VW:(h + 1) * VW],
                            pt[:, nh * NHW:(nh + 1) * NHW],
                            start=(kt == 0), stop=(kt == ST - 1))
                    if kt == ST - 1:
                        sched_norm(i, h)

                def sched_norm(i, h):
                    # Stagger the normalization chain across subsequent chain
                    # slots so each stage's deps are long-resolved before its
                    # strict-FIFO engine queue reaches it (no head-of-line
                    # blocking of the pt pipeline).
                    oaps = oaps_t.pop(h)
                    state = {}

                    def st_rc():
                        rc = rc_pool.tile([VW, S], f32, name="rc")
                        nc.vector.tensor_copy(rc[C:C + 1, :], oaps[C:C + 1, :])
                        state["rc"] = rc

                    def st_hop():
                        rc0 = rc0_pool.tile([1, S], f32, name="rc0")
                        nc.sync.dma_start(rc0[:], state["rc"][C:C + 1, :])
                        state["rc0"] = rc0

                    def st_recip():
                        rcv = rc0_pool.tile([1, S], f32, name="rcv", tag="rcv")
                        nc.vector.reciprocal_approx_fast(rcv[:], state["rc0"][:])
                        state["rcv"] = rcv

                    def st_bcast():
                        rcb = rcb_pool.tile([C, S], f32, name="rcb")
                        nc.gpsimd.partition_broadcast(rcb[:], state["rcv"][:])
                        state["rcb"] = rcb

                    def st_norm():
                        hp = h // 2
                        if h % 2 == 0:
                            nc.vector.tensor_mul(oT2_sb[0:C, hp, :],
                                                 oaps[0:C, :], state["rcb"][:])
                        else:
                            oTs = oshift_pool.tile([C, S], bf16, name="oTs")
                            nc.vector.tensor_mul(oTs[:], oaps[0:C, :],
                                                 state["rcb"][:])
                            nc.sync.dma_start(oT2_sb[C:P, hp, :], oTs[:])

                    for off, fn in ((0, st_rc), (1, st_hop), (2, st_recip),
                                    (3, st_bcast), (5, st_norm)):
                        events.setdefault(i + off, []).append(fn)

                chains = [(h, kt) for h in range(HG) for kt in range(ST)]
                LAG = 3
                n = len(chains)
                for i in range(n + LAG + 6):
                    if i < n:
                        emit_scores(i, *chains[i])
                    if LAG <= i < n + LAG:
                        emit_av(i, *chains[i - LAG])
                    for fn in events.pop(i, ()):
                        fn()
                assert not events and not pts and not oaps_t

            # ---- phase C: output projection (row-parallel partial) ----
            with tc.tile_pool(name="outsb", bufs=3) as out_pool, \
                 tc.tile_pool(name="psOut", bufs=2, space="PSUM") as psOut:
                for qt in range(ST):
                    for nh in range(2):
                        pso = psOut.tile([P, NHW], f32, name="pso")
                        for hp in range(F // P):
                            nc.tensor.matmul(
                                pso[:],
                                oT2_sb[:, hp, qt * P:(qt + 1) * P],
                                wo2_sb[:, hp, nh * NHW:(nh + 1) * NHW],
                                start=(hp == 0), stop=(hp == F // P - 1))
                        osb = out_pool.tile([P, NHW], bf16, name="osb")
                        nc.scalar.copy(osb[:], pso[:])
                        nc.sync.dma_start(
                            outp[qt * P:(qt + 1) * P,
                                 nh * NHW:(nh + 1) * NHW],
                            osb[:])

    nc.compile()
    return nc


def make_in_maps(x, bias, attention_mask, Wq, Wk, Wv, Wo):
    import ml_dtypes
    bf = ml_dtypes.bfloat16
    scale = 1.0 / math.sqrt(C)
    wq_scaled = (np.asarray(Wq) * scale).astype(bf)
    x = np.asarray(x)
    bias = np.asarray(bias)
    wk16 = np.asarray(Wk).astype(bf)
    wv16 = np.asarray(Wv).astype(bf)
    wo16 = np.asarray(Wo).astype(bf)
    in_maps = []
    for c in range(N_CORES):
        b, hg = c // 2, c % 2
        fs = slice(hg * F, (hg + 1) * F)
        in_maps.append({
            "xT": np.ascontiguousarray(x[b].T.astype(bf)),
            "wq": np.ascontiguousarray(wq_scaled[:, fs]),
            "wk": np.ascontiguousarray(wk16[:, fs]),
            "wv": np.ascontiguousarray(wv16[:, fs]),
            "wo": np.ascontiguousarray(wo16[fs, :]),
            "expb": np.ascontiguousarray(
                np.exp(bias[b, hg * HG:(hg + 1) * HG].astype(np.float32))
                .transpose(0, 2, 1).astype(bf)),
            "maskf": np.asarray(attention_mask)[b].astype(np.float32),
        })
    return in_maps


_NC_CACHE = []


def get_program():
    if not _NC_CACHE:
        _NC_CACHE.append(build_program())
    return _NC_CACHE[0]


def run(in_maps, trace=False, **kw):
    nc = get_program()
    return run_bass_kernel_spmd(nc, in_maps, core_ids=list(range(N_CORES)),
                                trace=trace, **kw)


def kernel(x, bias, attention_mask, Wq, Wk, Wv, Wo, bo):
    in_maps = make_in_maps(x, bias, attention_mask, Wq, Wk, Wv, Wo)
    res = run(in_maps)
    out = np.empty((B, S, C_IN), dtype=np.float32)
    for b in range(B):
        out[b] = (res.results[2 * b]["outp"].astype(np.float32)
                  + res.results[2 * b + 1]["outp"].astype(np.float32)
                  + np.asarray(bo).astype(np.float32))
    return out


# revision 11
# speedup vs baseline: 1.3830x; 1.3095x over previous
"""Trainium2 Bass kernel for CusMultiHeadAttention.

Shapes (hardcoded): x (4,1024,1024) f32, bias (4,16,1024,1024) f32,
attention_mask (4,1024) i32, Wq/Wk/Wv (1024,1024), Wo (1024,1024), bo (1024,).

Sharding: 8 cores = 4 batches x 2 head-groups (8 heads each).
Wq/Wk/Wv column-parallel, Wo row-parallel (host sums the pair partials + bo).

Differences from the first working version (292us), all aimed at keeping the
PE tensor engine dense so the HAM clock gate stays at 2.4GHz:
  - bias is sent as exp(bias) (bf16); the f32 psum bias-add (DVE 1x mode)
    becomes a bf16 x bf16 multiply after exp (DVE 2x_1P mode).
  - phase B is software-pipelined: scores for chain i issue ahead of the
    AV matmuls for chain i-2, so the PE queue never head-of-line blocks
    on the exp/mul chain.
  - V projection runs first (kt-outer, 4 psum groups x 2 passes) so the PE
    starts as soon as the first xT/wv chunks land; QK projections follow and
    phase B scores chain directly behind them.
  - output projection packs head pairs on 128 partitions (K=128 matmuls);
    odd heads' oT is partition-shifted 0:64 -> 64:128 via SBUF-SBUF DMA.
  - outp partial is bf16 (host accumulates in f32).
"""

import sys

if "/opt/trn_rl_repo" not in sys.path:
    sys.path.insert(0, "/opt/trn_rl_repo")

import math
from contextlib import ExitStack

import numpy as np

import concourse.mybir as mybir
import concourse.tile as tile
from concourse import bacc
from concourse.alu_op_type import AluOpType
from concourse.bass_utils import run_bass_kernel_spmd

B, S, C_IN = 4, 1024, 1024
N_HEAD, C = 16, 64
N_CORES = 8
HG = 8  # heads per core
F = HG * C  # 512 local features
P = 128
KT = C_IN // P  # 8 contraction tiles for projections
ST = S // P  # 8 seq tiles
VW = C + 1  # 65: v columns + ones-column
NHW = 512  # psum bank width in f32

f32 = mybir.dt.float32
bf16 = mybir.dt.bfloat16

# gpsimd runs ONLY partition_broadcast: mixing op types on the Pool engine
# forces MODIFY_POOL_CONFIG ucode reloads (~10us stalls observed in trace).


def build_program():
    nc = bacc.Bacc("TRN2", target_bir_lowering=False, debug=False,
                   num_devices=N_CORES)

    xT = nc.dram_tensor("xT", (C_IN, S), bf16, kind="ExternalInput").ap()
    wq = nc.dram_tensor("wq", (C_IN, F), bf16, kind="ExternalInput").ap()
    wk = nc.dram_tensor("wk", (C_IN, F), bf16, kind="ExternalInput").ap()
    wv = nc.dram_tensor("wv", (C_IN, F), bf16, kind="ExternalInput").ap()
    wo = nc.dram_tensor("wo", (F, C_IN), bf16, kind="ExternalInput").ap()
    expb = nc.dram_tensor("expb", (HG, S, S), bf16, kind="ExternalInput").ap()
    maskf = nc.dram_tensor("maskf", (S,), f32, kind="ExternalInput").ap()
    outp = nc.dram_tensor("outp", (S, C_IN), bf16, kind="ExternalOutput").ap()

    with tile.TileContext(nc) as tc:
        with ExitStack() as ctx:
            persist = ctx.enter_context(tc.tile_pool(name="persist", bufs=1))
            mask_sb = persist.tile([P, ST], f32)
            nc.sync.dma_start(mask_sb[:], maskf.rearrange("(t p) -> p t", p=P))
            ones_sb = persist.tile([P, HG, 1], f32)
            nc.vector.memset(ones_sb[:], 1.0)
            v_sb = persist.tile([P, ST, HG * VW], bf16)
            qT_sb = persist.tile([P, F // P, S], bf16)
            kT_sb = persist.tile([P, F // P, S], bf16)
            # head-pair rows: partitions 0:64 = even head, 64:128 = odd head
            # of pair hp; matches wo rows hp*128:(hp+1)*128.
            wo2_sb = persist.tile([P, F // P, C_IN], bf16)
            nc.sync.dma_start(
                wo2_sb[:], wo.rearrange("(h j) n -> j h n", j=P))
            oT2_sb = persist.tile([P, F // P, S], bf16)

            # expb pool opened early so its DMAs prefetch during phase A
            expb_pool = ctx.enter_context(tc.tile_pool(name="expb", bufs=12))

            # ---- phase A: projections ----
            with tc.tile_pool(name="phaseA", bufs=1) as pa:
                xT_sb = pa.tile([P, KT, S], bf16)
                wv_sb = pa.tile([P, KT, F], bf16)
                wq_sb = pa.tile([P, KT, F], bf16)
                wk_sb = pa.tile([P, KT, F], bf16)
                for kt in range(KT):
                    nc.sync.dma_start(xT_sb[:, kt, :], xT[kt * P:(kt + 1) * P, :])
                    nc.sync.dma_start(wv_sb[:, kt, :], wv[kt * P:(kt + 1) * P, :])
                for kt in range(KT):
                    nc.sync.dma_start(wq_sb[:, kt, :], wq[kt * P:(kt + 1) * P, :])
                    nc.sync.dma_start(wk_sb[:, kt, :], wk[kt * P:(kt + 1) * P, :])

                # v natural (seq on partitions), kt-outer so the PE starts on
                # the first loaded chunk; 2 passes of 4 seq-tiles (4 psum banks)
                with tc.tile_pool(name="psV", bufs=1, space="PSUM") as psV:
                    for half in range(2):
                        mts = range(half * 4, half * 4 + 4)
                        psv_t = {mt: psV.tile([P, F], f32, name=f"psv{mt % 4}")
                                 for mt in mts}
                        for kt in range(KT):
                            for mt in mts:
                                nc.tensor.matmul(
                                    psv_t[mt][:],
                                    xT_sb[:, kt, mt * P:(mt + 1) * P],
                                    wv_sb[:, kt, :],
                                    start=(kt == 0), stop=(kt == KT - 1))
                        for mt in mts:
                            m_col = mask_sb[:, mt:mt + 1]
                            v_view = v_sb[:, mt, :].rearrange(
                                "p (h c) -> p h c", c=VW)
                            nc.vector.tensor_scalar_mul(
                                v_view[:, :, 0:C],
                                psv_t[mt].rearrange("p (h c) -> p h c", c=C),
                                m_col)
                            nc.vector.tensor_scalar_mul(
                                v_view[:, :, C:C + 1], ones_sb[:], m_col)

                # qT, kT (feature on partitions, seq free)
                with tc.tile_pool(name="psProj", bufs=2, space="PSUM") as psProj:
                    for mt in range(F // P):
                        for w_sb, dst in ((wq_sb, qT_sb), (wk_sb, kT_sb)):
                            ps = psProj.tile([P, S], f32, name="ps_proj")
                            for nh in range(2):
                                for kt in range(KT):
                                    nc.tensor.matmul(
                                        ps[:, nh * NHW:(nh + 1) * NHW],
                                        w_sb[:, kt, mt * P:(mt + 1) * P],
                                        xT_sb[:, kt, nh * NHW:(nh + 1) * NHW],
                                        start=(kt == 0), stop=(kt == KT - 1))
                            nc.scalar.copy(dst[:, mt, :], ps[:])

            # ---- phase B: attention, software-pipelined chains ----
            with tc.tile_pool(name="pS", bufs=3) as es_pool, \
                 tc.tile_pool(name="pT", bufs=6) as pt_pool, \
                 tc.tile_pool(name="rc", bufs=2) as rc_pool, \
                 tc.tile_pool(name="rc0", bufs=4) as rc0_pool, \
                 tc.tile_pool(name="rcb", bufs=2) as rcb_pool, \
                 tc.tile_pool(name="oshift", bufs=2) as oshift_pool, \
                 tc.tile_pool(name="psS", bufs=2, space="PSUM") as psS, \
                 tc.tile_pool(name="psO", bufs=2, space="PSUM") as psO:

                pts = {}
                oaps_t = {}
                events = {}  # chain index -> [callables] staggered norm stages

                def emit_scores(i, h, kt):
                    po = (h % 2) * C
                    mt_h = h // 2
                    eb = expb_pool.tile([P, S], bf16, name="eb")
                    nc.sync.dma_start(eb[:], expb[h, kt * P:(kt + 1) * P, :])
                    ps_s = psS.tile([P, S], f32, name="ps_s")
                    for nh in range(2):
                        nc.tensor.matmul(
                            ps_s[:, nh * NHW:(nh + 1) * NHW],
                            kT_sb[po:po + C, mt_h, kt * P:(kt + 1) * P],
                            qT_sb[po:po + C, mt_h, nh * NHW:(nh + 1) * NHW],
                            start=True, stop=True)
                    es = es_pool.tile([P, S], bf16, name="es")
                    nc.scalar.activation(es[:], ps_s[:],
                                         mybir.ActivationFunctionType.Exp)
                    pt = pt_pool.tile([P, S], bf16, name="pt")
                    nc.vector.tensor_mul(pt[:], es[:], eb[:])
                    pts[(h, kt)] = pt

                def emit_av(i, h, kt):
                    if kt == 0:
                        oaps_t[h] = psO.tile([VW, S], f32, name="oaug")
                    oaps = oaps_t[h]
                    pt = pts.pop((h, kt))
                    for nh in range(2):
                        nc.tensor.matmul(
                            oaps[:, nh * NHW:(nh + 1) * NHW],
                            v_sb[:, kt, h * VW:(h + 1) * VW],
                            pt[:, nh * NHW:(nh + 1) * NHW],
                            start=(kt == 0), stop=(kt == ST - 1))
                    if kt == ST - 1:
                        sched_norm(i, h)

                def sched_norm(i, h):
                    # Stagger the normalization chain across subsequent chain
                    # slots so each stage's deps are long-resolved before its
                    # strict-FIFO engine queue reaches it (no head-of-line
                    # blocking of the pt pipeline).
                    oaps = oaps_t.pop(h)
                    state = {}

                    def st_rc():
                        rc = rc_pool.tile([VW, S], f32, name="rc")
                        nc.vector.tensor_copy(rc[C:C + 1, :], oaps[C:C + 1, :])
                        state["rc"] = rc

                    def st_hop():
                        rc0 = rc0_pool.tile([1, S], f32, name="rc0")
                        nc.sync.dma_start(rc0[:], state["rc"][C:C + 1, :])
                        state["rc0"] = rc0

                    def st_recip():
                        rcv = rc0_pool.tile([1, S], f32, name="rcv", tag="rcv")
                        nc.vector.reciprocal_approx_fast(rcv[:], state["rc0"][:])
                        state["rcv"] = rcv

                    def st_bcast():
                        rcb = rcb_pool.tile([C, S], f32, name="rcb")
                        nc.gpsimd.partition_broadcast(rcb[:], state["rcv"][:])
                        state["rcb"] = rcb

                    def st_norm():
                        hp = h // 2
                        if h % 2 == 0:
                            nc.vector.tensor_mul(oT2_sb[0:C, hp, :],
                                                 oaps[0:C, :], state["rcb"][:])
                        else:
                            oTs = oshift_pool.tile([C, S], bf16, name="oTs")
                            nc.vector.tensor_mul(oTs[:], oaps[0:C, :],
                                                 state["rcb"][:])
                            nc.sync.dma_start(oT2_sb[C:P, hp, :], oTs[:])

                    for off, fn in ((0, st_rc), (1, st_hop), (2, st_recip),
                                    (3, st_bcast), (5, st_norm)):
                        events.setdefault(i + off, []).append(fn)

                chains = [(h, kt) for h in range(HG) for kt in range(ST)]
                LAG = 3
                n = len(chains)
                for i in range(n + LAG + 6):
                    if i < n:
                        emit_scores(i, *chains[i])
                    if LAG <= i < n + LAG:
                        emit_av(i, *chains[i - LAG])
                    for fn in events.pop(i, ()):
                        fn()
                assert not events and not pts and not oaps_t

            # ---- phase C: output projection (row-parallel partial) ----
            with tc.tile_pool(name="outsb", bufs=3) as out_pool, \
                 tc.tile_pool(name="psOut", bufs=2, space="PSUM") as psOut:
                for qt in range(ST):
                    for nh in range(2):
                        pso = psOut.tile([P, NHW], f32, name="pso")
                        for hp in range(F // P):
                            nc.tensor.matmul(
                                pso[:],
                                oT2_sb[:, hp, qt * P:(qt + 1) * P],
                                wo2_sb[:, hp, nh * NHW:(nh + 1) * NHW],
                                start=(hp == 0), stop=(hp == F // P - 1))
                        osb = out_pool.tile([P, NHW], bf16, name="osb")
                        nc.scalar.copy(osb[:], pso[:])
                        nc.sync.dma_start(
                            outp[qt * P:(qt + 1) * P,
                                 nh * NHW:(nh + 1) * NHW],
                            osb[:])

    nc.compile()
    return nc


def make_in_maps(x, bias, attention_mask, Wq, Wk, Wv, Wo):
    import ml_dtypes
    bf = ml_dtypes.bfloat16
    scale = 1.0 / math.sqrt(C)
    wq_scaled = (np.asarray(Wq) * scale).astype(bf)
    x = np.asarray(x)
    bias = np.asarray(bias)
    wk16 = np.asarray(Wk).astype(bf)
    wv16 = np.asarray(Wv).astype(bf)
    wo16 = np.asarray(Wo).astype(bf)
    in_maps = []
    for c in range(N_CORES):
        b, hg = c // 2, c % 2
        fs = slice(hg * F, (hg + 1) * F)
        in_maps.append({
            "xT": np.ascontiguousarray(x[b].T.astype(bf)),
            "wq": np.ascontiguousarray(wq_scaled[:, fs]),
            "wk": np.ascontiguousarray(wk16[:, fs]),
            "wv": np.ascontiguousarray(wv16[:, fs]),
            "wo": np.ascontiguousarray(wo16[fs, :]),
            "expb": np.ascontiguousarray(
                np.exp(bias[b, hg * HG:(hg + 1) * HG].astype(np.float32))
                .transpose(0, 2, 1).astype(bf)),
            "maskf": np.asarray(attention_mask)[b].astype(np.float32),
        })
    return in_maps


_NC_CACHE = []


def get_program():
    if not _NC_CACHE:
        _NC_CACHE.append(build_program())
    return _NC_CACHE[0]


def run(in_maps, trace=False, **kw):
    nc = get_program()
    return run_bass_kernel_spmd(nc, in_maps, core_ids=list(range(N_CORES)),
                                trace=trace, **kw)


def kernel(x, bias, attention_mask, Wq, Wk, Wv, Wo, bo):
    in_maps = make_in_maps(x, bias, attention_mask, Wq, Wk, Wv, Wo)
    res = run(in_maps)
    out = np.empty((B, S, C_IN), dtype=np.float32)
    for b in range(B):
        out[b] = (res.results[2 * b]["outp"].astype(np.float32)
                  + res.results[2 * b + 1]["outp"].astype(np.float32)
                  + np.asarray(bo).astype(np.float32))
    return out


# revision 20
# speedup vs baseline: 1.3889x; 1.0043x over previous
"""Trainium2 Bass kernel for CusMultiHeadAttention.

Shapes (hardcoded): x (4,1024,1024) f32, bias (4,16,1024,1024) f32,
attention_mask (4,1024) i32, Wq/Wk/Wv (1024,1024), Wo (1024,1024), bo (1024,).

Sharding: 8 cores = 4 batches x 2 head-groups (8 heads each).
Wq/Wk/Wv column-parallel, Wo row-parallel (host sums the pair partials + bo).

Differences from the first working version (292us), all aimed at keeping the
PE tensor engine dense so the HAM clock gate stays at 2.4GHz:
  - bias is sent as exp(bias) (bf16); the f32 psum bias-add (DVE 1x mode)
    becomes a bf16 x bf16 multiply after exp (DVE 2x_1P mode).
  - phase B is software-pipelined: scores for chain i issue ahead of the
    AV matmuls for chain i-2, so the PE queue never head-of-line blocks
    on the exp/mul chain.
  - V projection runs first (kt-outer, 4 psum groups x 2 passes) so the PE
    starts as soon as the first xT/wv chunks land; QK projections follow and
    phase B scores chain directly behind them.
  - output projection packs head pairs on 128 partitions (K=128 matmuls);
    odd heads' oT is partition-shifted 0:64 -> 64:128 via SBUF-SBUF DMA.
  - outp partial is bf16 (host accumulates in f32).
"""

import sys

if "/opt/trn_rl_repo" not in sys.path:
    sys.path.insert(0, "/opt/trn_rl_repo")

import math
from contextlib import ExitStack

import numpy as np

import concourse.mybir as mybir
import concourse.tile as tile
from concourse import bacc
from concourse.alu_op_type import AluOpType
from concourse.bass_utils import run_bass_kernel_spmd

B, S, C_IN = 4, 1024, 1024
N_HEAD, C = 16, 64
N_CORES = 8
HG = 8  # heads per core
F = HG * C  # 512 local features
P = 128
KT = C_IN // P  # 8 contraction tiles for projections
ST = S // P  # 8 seq tiles
VW = C + 1  # 65: v columns + ones-column
NHW = 512  # psum bank width in f32

f32 = mybir.dt.float32
bf16 = mybir.dt.bfloat16
f8 = mybir.dt.float8e4
DR = mybir.MatmulPerfMode.DoubleRow
# fp8 weight prescales (powers of 2): wq carries 1/sqrt(C) and x~N(0,1) is
# sent as-is; scores psum is SWQ*SWK too big, undone by the exp input scale.
SWQ, SWK, SWV = 1.0, 1.0, 1.0

# gpsimd runs ONLY partition_broadcast: mixing op types on the Pool engine
# forces MODIFY_POOL_CONFIG ucode reloads (~10us stalls observed in trace).


def build_program():
    nc = bacc.Bacc("TRN2", target_bir_lowering=False, debug=False,
                   num_devices=N_CORES)

    xT = nc.dram_tensor("xT", (C_IN, S), bf16, kind="ExternalInput").ap()
    wq = nc.dram_tensor("wq", (C_IN, F), bf16, kind="ExternalInput").ap()
    wk = nc.dram_tensor("wk", (C_IN, F), bf16, kind="ExternalInput").ap()
    wv = nc.dram_tensor("wv", (C_IN, F), bf16, kind="ExternalInput").ap()
    maskd = nc.dram_tensor("maskd", (S,), f32, kind="ExternalInput").ap()
    wo = nc.dram_tensor("wo", (F, C_IN), bf16, kind="ExternalInput").ap()
    expb = nc.dram_tensor("expb", (HG, S, S), bf16, kind="ExternalInput").ap()
    maskf = nc.dram_tensor("maskf", (S,), f32, kind="ExternalInput").ap()
    outp = nc.dram_tensor("outp", (S, C_IN), bf16, kind="ExternalOutput").ap()

    with tile.TileContext(nc) as tc:
        with ExitStack() as ctx:
            persist = ctx.enter_context(tc.tile_pool(name="persist", bufs=1))
            mask_sb = persist.tile([P, ST], f32)
            nc.sync.dma_start(mask_sb[:], maskf.rearrange("(t p) -> p t", p=P))
            maskd_sb = persist.tile([P, ST], f32)
            nc.sync.dma_start(maskd_sb[:],
                              maskd.rearrange("(t p) -> p t", p=P))
            ones_sb = persist.tile([P, HG, 1], f32)
            nc.vector.memset(ones_sb[:], 1.0)
            v_sb = persist.tile([P, ST, HG * VW], bf16)
            qT_sb = persist.tile([P, F // P, S], bf16)
            kT_sb = persist.tile([P, F // P, S], bf16)
            # head-pair rows: partitions 0:64 = even head, 64:128 = odd head
            # of pair hp; matches wo rows hp*128:(hp+1)*128.
            wo2_sb = persist.tile([P, F // P, C_IN], bf16)
            nc.sync.dma_start(
                wo2_sb[:], wo.rearrange("(h j) n -> j h n", j=P))
            oT2_sb = persist.tile([P, F // P, S], bf16)
            warm_bc = persist.tile([C, 1], f32)
            nc.gpsimd.partition_broadcast(warm_bc[:], ones_sb[0:1, 0, :])

            # expb pool opened early so its DMAs prefetch during phase A
            expb_pool = ctx.enter_context(tc.tile_pool(name="expb", bufs=12))

            # ---- phase A: projections ----
            with tc.tile_pool(name="phaseA", bufs=1) as pa:
                xT_sb = pa.tile([P, KT, S], bf16)
                wv_sb = pa.tile([P, KT, F], bf16)
                wq_sb = pa.tile([P, KT, F], bf16)
                wk_sb = pa.tile([P, KT, F], bf16)
                for kt in range(KT):
                    nc.sync.dma_start(xT_sb[:, kt, :], xT[kt * P:(kt + 1) * P, :])
                    nc.sync.dma_start(wv_sb[:, kt, :], wv[kt * P:(kt + 1) * P, :])
                for kt in range(KT):
                    nc.sync.dma_start(wq_sb[:, kt, :], wq[kt * P:(kt + 1) * P, :])
                    nc.sync.dma_start(wk_sb[:, kt, :], wk[kt * P:(kt + 1) * P, :])

                # v natural (seq on partitions), kt-outer so the PE starts on
                # the first loaded chunk; 2 passes of 4 seq-tiles (4 psum banks)
                with tc.tile_pool(name="psV", bufs=1, space="PSUM") as psV:
                    for half in range(2):
                        mts = range(half * 4, half * 4 + 4)
                        psv_t = {mt: psV.tile([P, F], f32, name=f"psv{mt % 4}")
                                 for mt in mts}
                        for kt in range(KT):
                            for mt in mts:
                                nc.tensor.matmul(
                                    psv_t[mt][:],
                                    xT_sb[:, kt, mt * P:(mt + 1) * P],
                                    wv_sb[:, kt, :],
                                    start=(kt == 0), stop=(kt == KT - 1))
                        for mt in mts:
                            m_col = mask_sb[:, mt:mt + 1]
                            v_view = v_sb[:, mt, :].rearrange(
                                "p (h c) -> p h c", c=VW)
                            nc.vector.tensor_scalar_mul(
                                v_view[:, :, 0:C],
                                psv_t[mt].rearrange("p (h c) -> p h c", c=C),
                                maskd_sb[:, mt:mt + 1])
                            nc.vector.tensor_scalar_mul(
                                v_view[:, :, C:C + 1], ones_sb[:], m_col)

                # qT, kT (feature on partitions, seq free)
                with tc.tile_pool(name="psProj", bufs=2, space="PSUM") as psProj:
                    for mt in range(F // P):
                        for w_sb, dst in ((wq_sb, qT_sb), (wk_sb, kT_sb)):
                            ps = psProj.tile([P, S], f32, name="ps_proj")
                            for nh in range(2):
                                for kt in range(KT):
                                    nc.tensor.matmul(
                                        ps[:, nh * NHW:(nh + 1) * NHW],
                                        w_sb[:, kt, mt * P:(mt + 1) * P],
                                        xT_sb[:, kt, nh * NHW:(nh + 1) * NHW],
                                        start=(kt == 0), stop=(kt == KT - 1))
                            nc.scalar.copy(dst[:, mt, :], ps[:])

            # ---- phase B: attention, software-pipelined chains ----
            with tc.tile_pool(name="pS", bufs=4) as es_pool, \
                 tc.tile_pool(name="pT", bufs=7) as pt_pool, \
                 tc.tile_pool(name="rc", bufs=2) as rc_pool, \
                 tc.tile_pool(name="rc0", bufs=4) as rc0_pool, \
                 tc.tile_pool(name="rcb", bufs=2) as rcb_pool, \
                 tc.tile_pool(name="oshift", bufs=2) as oshift_pool, \
                 tc.tile_pool(name="psS", bufs=2, space="PSUM") as psS, \
                 tc.tile_pool(name="psO", bufs=2, space="PSUM") as psO:

                pts = {}
                oaps_t = {}
                events = {}  # chain index -> [callables] staggered norm stages

                def emit_scores(i, h, kt):
                    po = (h % 2) * C
                    mt_h = h // 2
                    eb = expb_pool.tile([P, S], bf16, name="eb")
                    nc.sync.dma_start(eb[:], expb[h, kt * P:(kt + 1) * P, :])
                    ps_s = psS.tile([P, S], f32, name="ps_s")
                    for nh in range(2):
                        nc.tensor.matmul(
                            ps_s[:, nh * NHW:(nh + 1) * NHW],
                            kT_sb[po:po + C, mt_h, kt * P:(kt + 1) * P],
                            qT_sb[po:po + C, mt_h, nh * NHW:(nh + 1) * NHW],
                            start=True, stop=True)
                    es = es_pool.tile([P, S], bf16, name="es")
                    nc.scalar.activation(es[:], ps_s[:],
                                         mybir.ActivationFunctionType.Exp,
                                         scale=1.0 / (SWQ * SWK))
                    pt = pt_pool.tile([P, S], bf16, name="pt")
                    nc.vector.tensor_mul(pt[:], es[:], eb[:])
                    pts[(h, kt)] = pt

                def emit_av(i, h, kt):
                    if kt == 0:
                        oaps_t[h] = psO.tile([VW, S], f32, name="oaug")
                    oaps = oaps_t[h]
                    pt = pts.pop((h, kt))
                    for nh in range(2):
                        nc.tensor.matmul(
                            oaps[:, nh * NHW:(nh + 1) * NHW],
                            v_sb[:, kt, h * VW:(h + 1) * VW],
                            pt[:, nh * NHW:(nh + 1) * NHW],
                            start=(kt == 0), stop=(kt == ST - 1))
                    if kt == ST - 1:
                        sched_norm(i, h)

                def sched_norm(i, h):
                    # Stagger the normalization chain across subsequent chain
                    # slots so each stage's deps are long-resolved before its
                    # strict-FIFO engine queue reaches it (no head-of-line
                    # blocking of the pt pipeline).
                    oaps = oaps_t.pop(h)
                    state = {}

                    def st_rc():
                        rc = rc_pool.tile([VW, S], f32, name="rc")
                        nc.vector.tensor_copy(rc[C:C + 1, :], oaps[C:C + 1, :])
                        state["rc"] = rc

                    def st_hop():
                        rc0 = rc0_pool.tile([1, S], f32, name="rc0")
                        nc.sync.dma_start(rc0[:], state["rc"][C:C + 1, :])
                        state["rc0"] = rc0

                    def st_recip():
                        rcv = rc0_pool.tile([1, S], f32, name="rcv", tag="rcv")
                        nc.vector.reciprocal_approx_fast(rcv[:], state["rc0"][:])
                        state["rcv"] = rcv

                    def st_bcast():
                        rcb = rcb_pool.tile([C, S], f32, name="rcb")
                        nc.gpsimd.partition_broadcast(rcb[:], state["rcv"][:])
                        state["rcb"] = rcb

                    def st_norm():
                        hp = h // 2
                        if h % 2 == 0:
                            nc.vector.tensor_mul(oT2_sb[0:C, hp, :],
                                                 oaps[0:C, :], state["rcb"][:])
                        else:
                            oTs = oshift_pool.tile([C, S], bf16, name="oTs")
                            nc.vector.tensor_mul(oTs[:], oaps[0:C, :],
                                                 state["rcb"][:])
                            nc.sync.dma_start(oT2_sb[C:P, hp, :], oTs[:])

                    for off, fn in ((0, st_rc), (1, st_hop), (2, st_recip),
                                    (3, st_bcast), (5, st_norm)):
                        events.setdefault(i + off, []).append(fn)

                chains = [(h, kt) for h in range(HG) for kt in range(ST)]
                LAG = 4
                n = len(chains)
                for i in range(n + LAG + 6):
                    if i < n:
                        emit_scores(i, *chains[i])
                    if LAG <= i < n + LAG:
                        emit_av(i, *chains[i - LAG])
                    for fn in events.pop(i, ()):
                        fn()
                assert not events and not pts and not oaps_t

            # ---- phase C: output projection (row-parallel partial) ----
            with tc.tile_pool(name="outsb", bufs=3) as out_pool, \
                 tc.tile_pool(name="psOut", bufs=2, space="PSUM") as psOut:
                for qt in range(ST):
                    for nh in range(2):
                        pso = psOut.tile([P, NHW], f32, name="pso")
                        for hp in range(F // P):
                            nc.tensor.matmul(
                                pso[:],
                                oT2_sb[:, hp, qt * P:(qt + 1) * P],
                                wo2_sb[:, hp, nh * NHW:(nh + 1) * NHW],
                                start=(hp == 0), stop=(hp == F // P - 1))
                        osb = out_pool.tile([P, NHW], bf16, name="osb")
                        nc.scalar.copy(osb[:], pso[:])
                        nc.sync.dma_start(
                            outp[qt * P:(qt + 1) * P,
                                 nh * NHW:(nh + 1) * NHW],
                            osb[:])

    nc.compile()
    return nc


def make_in_maps(x, bias, attention_mask, Wq, Wk, Wv, Wo):
    import ml_dtypes
    bf = ml_dtypes.bfloat16
    f8n = ml_dtypes.float8_e4m3
    scale = 1.0 / math.sqrt(C)
    wq8 = (np.asarray(Wq) * scale).astype(bf)
    x = np.asarray(x)
    bias = np.asarray(bias)
    wk8 = np.asarray(Wk).astype(bf)
    wv8 = np.asarray(Wv).astype(bf)
    wo16 = np.asarray(Wo).astype(bf)
    mask = np.asarray(attention_mask)
    in_maps = []
    for c in range(N_CORES):
        b, hg = c // 2, c % 2
        fs = slice(hg * F, (hg + 1) * F)
        in_maps.append({
            "xT": np.ascontiguousarray(x[b].T.astype(bf)),
            "wq": np.ascontiguousarray(wq8[:, fs]),
            "wk": np.ascontiguousarray(wk8[:, fs]),
            "wv": np.ascontiguousarray(wv8[:, fs]),
            "wo": np.ascontiguousarray(wo16[fs, :]),
            "expb": np.ascontiguousarray(
                np.exp(bias[b, hg * HG:(hg + 1) * HG].astype(np.float32))
                .transpose(0, 2, 1).astype(bf)),
            "maskf": mask[b].astype(np.float32),
            "maskd": (mask[b].astype(np.float32) / SWV),
        })
    return in_maps


_NC_CACHE = []


def get_program():
    if not _NC_CACHE:
        _NC_CACHE.append(build_program())
    return _NC_CACHE[0]


def run(in_maps, trace=False, **kw):
    nc = get_program()
    return run_bass_kernel_spmd(nc, in_maps, core_ids=list(range(N_CORES)),
                                trace=trace, **kw)


def kernel(x, bias, attention_mask, Wq, Wk, Wv, Wo, bo):
    in_maps = make_in_maps(x, bias, attention_mask, Wq, Wk, Wv, Wo)
    res = run(in_maps)
    out = np.empty((B, S, C_IN), dtype=np.float32)
    for b in range(B):
        out[b] = (res.results[2 * b]["outp"].astype(np.float32)
                  + res.results[2 * b + 1]["outp"].astype(np.float32)
                  + np.asarray(bo).astype(np.float32))
    return out
